# revision 1
# baseline (speedup 1.0000x reference)
"""Trainium2 Bass kernel for the mu/sigma Conv2d problem.

Math (per reference):
  mu_y    = conv(mu_x, W) + bias
  sigma_y = (softplus(w_sigma) * (conv(sigma_x, ones) + conv(mu_x^2, ones))
             + conv(sigma_x, W^2)) * 1e-3

Shapes: mu_x/sigma_x [16,128,96,96], W [256,128,5,5], bias [256],
w_sigma [256,1].  Outputs [16,256,92,92] (VALID conv).

Strategy: data-parallel over batch across 8 NeuronCores (2 images/core).
Each conv is a direct conv: for each 5-row output block, 25 accumulating
fp32r matmuls (contraction over C=128 in partitions) into one PSUM bank.
The box-filter term conv(sigma_x + mu_x^2, ones[1,C,5,5]) is computed
cheaply per image: channel-sum via ones-matmul -> [96,96] plane, vertical
5-box via a banded matmul, horizontal 5-box on the vector engine; the
per-output-channel softplus scale is folded in as one extra rank-1 matmul
accumulated into each sigma PSUM group.  The 1e-3 scale is folded into
W^2 and softplus(w_sigma) host-side; bias is added during PSUM eviction
on the scalar engine.
"""

import numpy as np

import concourse.bacc as bacc
import concourse.tile as tile
from concourse import mybir
from concourse.bass_utils import run_bass_kernel_spmd

F32 = mybir.dt.float32
F32R = mybir.dt.float32r
BF16 = mybir.dt.bfloat16
FP16 = mybir.dt.float16

B, C, O, H, W_IN, KK = 16, 128, 256, 96, 96, 5
HO = WO = 92
NCORES = 8
BPC = B // NCORES          # images per core
OCH = O // 128             # output-channel chunks
RB = 5                     # output rows per PSUM group
NPIX = RB * WO             # 460 <= 512 (one fp32 PSUM bank)

# 19 output row blocks; the last starts at 87 so it stays full-height
# (rows 87..91), overlapping rows 87..89 of the previous block (benign
# double-write of identical values).  Full N=460 keeps fp32r at rate 1.
BLOCK_STARTS = [5 * i for i in range(18)] + [HO - RB]
# channel-sum chunks over the 96 input rows: 19 x 5 rows + one final
# 5-row chunk starting at 91 (rows 91..95, overlap rows 91..94).
CS_STARTS = [5 * i for i in range(19)] + [H - RB]
# row-block sets: all blocks in a set accumulate concurrently in distinct
# PSUM banks so one LDWEIGHTS serves the whole set (5x fewer weight loads)
BLOCK_SETS = [BLOCK_STARTS[i : i + 5] for i in range(0, len(BLOCK_STARTS), 5)]

_CACHE = {}


def _build(iters=1):
    key = ("nc", iters)
    if key in _CACHE:
        return _CACHE[key]

    nc = bacc.Bacc(None)
    mu_d = nc.dram_tensor("mu", [BPC, C, H, W_IN], F32R, kind="ExternalInput")
    sg_d = nc.dram_tensor("sg", [BPC, C, H, W_IN], F32R, kind="ExternalInput")
    wmu_d = nc.dram_tensor("wmu", [C, OCH, KK * KK, 128], F32R, kind="ExternalInput")
    wsg_d = nc.dram_tensor("wsg", [C, OCH, KK * KK, 128], F32R, kind="ExternalInput")
    bias_d = nc.dram_tensor("bias", [128, OCH], F32, kind="ExternalInput")
    sp_d = nc.dram_tensor("sp", [1, O], F32, kind="ExternalInput")
    band_d = nc.dram_tensor("band", [H, HO], F32, kind="ExternalInput")
    muy_d = nc.dram_tensor("muy", [BPC, O, HO, WO], F32, kind="ExternalOutput")
    sgy_d = nc.dram_tensor("sgy", [BPC, O, HO, WO], F32, kind="ExternalOutput")

    with tile.TileContext(nc) as tc:
        with (
            tc.tile_pool(name="consts", bufs=1) as consts,
            tc.tile_pool(name="imgs", bufs=1) as imgs,
            tc.tile_pool(name="boxs", bufs=2) as boxs,
            tc.tile_pool(name="ufc", bufs=3) as ufc,
            tc.tile_pool(name="bfc", bufs=4) as bfc,
            tc.tile_pool(name="stag_mu", bufs=3) as stag_mu,
            tc.tile_pool(name="stag_sg", bufs=3) as stag_sg,
            tc.tile_pool(name="ps_conv", bufs=6, space="PSUM") as ps_conv,
            tc.tile_pool(name="ps_u", bufs=1, space="PSUM") as ps_u,
            tc.tile_pool(name="ps_v", bufs=1, space="PSUM") as ps_v,
        ):
            wmu_sb = consts.tile([C, OCH, KK * KK, 128], F32R)
            wsg_sb = consts.tile([C, OCH, KK * KK, 128], F32R)
            bias_sb = consts.tile([128, OCH], F32)
            sp_sb = consts.tile([1, O], F32)
            band_sb = consts.tile([H, HO], F32)
            ones_col = consts.tile([C, 1], FP16)
            nc.sync.dma_start(wmu_sb[:], wmu_d[:])
            nc.sync.dma_start(wsg_sb[:], wsg_d[:])
            nc.sync.dma_start(bias_sb[:], bias_d[:])
            nc.sync.dma_start(sp_sb[:], sp_d[:])
            nc.sync.dma_start(band_sb[:], band_d[:])
            nc.vector.memset(ones_col[:], 1.0)

            import contextlib

            loop_cm = tc.For_i(0, iters, 1) if iters > 1 else contextlib.nullcontext()
            with loop_cm:
              for img in range(BPC):
                  mu_sb = imgs.tile([C, H, W_IN], F32R, tag="mu")
                  sg_sb = imgs.tile([C, H, W_IN], F32R, tag="sg")
                  nc.sync.dma_start(mu_sb[:], mu_d[img])
                  nc.sync.dma_start(sg_sb[:], sg_d[img])

                  # t = sigma_x + mu_x^2 (bf16 is plenty: it only feeds the
                  # 3200-term box sums, where rounding error averages out)
                  t_bf = imgs.tile([C, H, W_IN], FP16, tag="t")
                  nc.vector.tensor_mul(t_bf[:], mu_sb[:].bitcast(F32), mu_sb[:].bitcast(F32))
                  nc.vector.tensor_add(t_bf[:], t_bf[:], sg_sb[:].bitcast(F32))

                  def conv_group(x_sb, w_sb, r0, och, extra, evict):
                      ps = ps_conv.tile([128, RB, WO], F32, tag="ps")
                      ki = 0
                      for kh in range(KK):
                          for kw in range(KK):
                              nc.tensor.matmul(
                                  ps[:],
                                  w_sb[:, och, ki, :],
                                  x_sb[:, r0 + kh : r0 + kh + RB, kw : kw + WO],
                                  start=(ki == 0),
                                  stop=False,
                              )
                              ki += 1
                      extra(ps)
                      evict(ps)

                  # ---- mu conv, first output-channel chunk ----
                  def mu_extra(ps, och):
                      # close the accumulation group: bias is added at
                      # eviction time on the scalar engine
                      pass

                  def mu_evict(ps, r0, och):
                      st = stag_mu.tile([128, RB, WO], F32, tag="st")
                      nc.scalar.add(st[:], ps[:], bias_sb[:, och : och + 1])
                      nc.sync.dma_start(
                          muy_d[img, och * 128 : (och + 1) * 128, r0 : r0 + RB, :],
                          st[:],
                      )

                  def sg_evict(ps, r0, och):
                      st = stag_sg.tile([128, RB, WO], F32, tag="st")
                      nc.vector.tensor_copy(st[:], ps[:])
                      nc.sync.dma_start(
                          sgy_d[img, och * 128 : (och + 1) * 128, r0 : r0 + RB, :],
                          st[:],
                      )

                  def mu_groups(och):
                      for blocks in BLOCK_SETS:
                          pss = [
                              (r0, ps_conv.tile([128, RB, WO], F32, tag="ps", name=f"ps{r0}"))
                              for r0 in blocks
                          ]
                          for ki in range(KK * KK):
                              kh, kw = divmod(ki, KK)
                              for r0, ps in pss:
                                  nc.tensor.matmul(
                                      ps[:],
                                      wmu_sb[:, och, ki, :],
                                      mu_sb[:, r0 + kh : r0 + kh + RB, kw : kw + WO],
                                      start=(ki == 0),
                                      stop=(ki == KK * KK - 1),
                                  )
                          for r0, ps in pss:
                              mu_evict(ps, r0, och)

                  mu_groups(0)

                  # ---- box-filter pipeline (between the two mu chunks so the
                  # PE never waits on it) ----
                  u2d = boxs.tile([H, W_IN], F32, tag="u2d")
                  for cs in CS_STARTS:
                      ups = ps_u.tile([1, RB * W_IN], F32, tag="ups")
                      nc.tensor.matmul(
                          ups[:],
                          ones_col[:],
                          t_bf[:, cs : cs + RB, :],
                          start=True,
                          stop=True,
                      )
                      uf = ufc.tile([1, RB * W_IN], F32, tag="uf")
                      nc.scalar.copy(uf[:], ups[:])
                      nc.sync.dma_start(u2d[cs : cs + RB, :], uf[:])

                  vb_ps = ps_v.tile([HO, W_IN], F32, tag="vb")
                  nc.tensor.matmul(
                      vb_ps[:],
                      band_sb[:],
                      u2d[:],
                      start=True,
                      stop=True,
                  )
                  vb_sb = boxs.tile([HO, W_IN], F32, tag="vbs")
                  nc.vector.tensor_copy(vb_sb[:], vb_ps[:])
                  box2d = boxs.tile([HO, WO], F32, tag="box")
                  nc.vector.tensor_add(box2d[:], vb_sb[:, 0:WO], vb_sb[:, 1 : 1 + WO])
                  for kw in (2, 3, 4):
                      nc.vector.tensor_add(box2d[:], box2d[:], vb_sb[:, kw : kw + WO])

                  mu_groups(1)

                  # ---- sigma conv (both chunks); the softplus term enters as
                  # one rank-1 matmul accumulated into the same PSUM group ----
                  for och in range(OCH):
                      for blocks in BLOCK_SETS:
                          bfs = []
                          for r0 in blocks:
                              bf = bfc.tile([1, RB * WO], F32, tag="bf", name=f"bf{r0}")
                              nc.sync.dma_start(bf[:], box2d[r0 : r0 + RB, :])
                              bfs.append(bf)
                          pss = [
                              (r0, ps_conv.tile([128, RB, WO], F32, tag="ps", name=f"ps{r0}"))
                              for r0 in blocks
                          ]
                          for ki in range(KK * KK):
                              kh, kw = divmod(ki, KK)
                              for r0, ps in pss:
                                  nc.tensor.matmul(
                                      ps[:],
                                      wsg_sb[:, och, ki, :],
                                      sg_sb[:, r0 + kh : r0 + kh + RB, kw : kw + WO],
                                      start=(ki == 0),
                                      stop=False,
                                  )
                          for (r0, ps), bf in zip(pss, bfs):
                              nc.tensor.matmul(
                                  ps[:],
                                  sp_sb[0:1, och * 128 : (och + 1) * 128],
                                  bf[:],
                                  start=False,
                                  stop=True,
                              )
                          for r0, ps in pss:
                              sg_evict(ps, r0, och)

    nc.compile()
    _CACHE[key] = nc
    return nc


def _host_prep(mu_x, sigma_x, W, bias, w_sigma):
    W = np.asarray(W, dtype=np.float32)
    bias = np.asarray(bias, dtype=np.float32)
    w_sigma = np.asarray(w_sigma, dtype=np.float32)

    # [o, c, kh, kw] -> [c, och, k, o_in]
    w4 = W.reshape(OCH, 128, C, KK * KK)
    wmu = np.ascontiguousarray(w4.transpose(2, 0, 3, 1))
    w2 = (W.astype(np.float64) ** 2 * 1e-3).astype(np.float32)
    wsg = np.ascontiguousarray(
        w2.reshape(OCH, 128, C, KK * KK).transpose(2, 0, 3, 1)
    )
    bias_arr = np.ascontiguousarray(bias.reshape(OCH, 128).T)
    sp = np.log(1.0 + np.exp(np.maximum(w_sigma.astype(np.float64), -88.0)))
    sp_row = np.ascontiguousarray((sp[:, 0] * 1e-3).astype(np.float32)[None, :])
    band = np.zeros((H, HO), dtype=np.float32)
    for y2 in range(HO):
        band[y2 : y2 + KK, y2] = 1.0
    return wmu, wsg, bias_arr, sp_row, band


def kernel(mu_x, sigma_x, W, bias, w_sigma):
    mu_x = np.asarray(mu_x, dtype=np.float32)
    sigma_x = np.asarray(sigma_x, dtype=np.float32)
    wmu, wsg, bias_arr, sp_row, band = _host_prep(mu_x, sigma_x, W, bias, w_sigma)

    nc = _build()
    in_maps = []
    for c in range(NCORES):
        in_maps.append(
            {
                "mu": mu_x[c * BPC : (c + 1) * BPC],
                "sg": sigma_x[c * BPC : (c + 1) * BPC],
                "wmu": wmu,
                "wsg": wsg,
                "bias": bias_arr,
                "sp": sp_row,
                "band": band,
            }
        )
    res = run_bass_kernel_spmd(nc, in_maps, core_ids=list(range(NCORES)))
    mu_y = np.concatenate([res.results[c]["muy"] for c in range(NCORES)], axis=0)
    sigma_y = np.concatenate([res.results[c]["sgy"] for c in range(NCORES)], axis=0)
    return mu_y.astype(np.float32), sigma_y.astype(np.float32)



# revision 7
# speedup vs baseline: 1.1079x; 1.1079x over previous
"""Trainium2 Bass kernel for the mu/sigma Conv2d problem.

Math (per reference):
  mu_y    = conv(mu_x, W) + bias
  sigma_y = (softplus(w_sigma) * (conv(sigma_x, ones) + conv(mu_x^2, ones))
             + conv(sigma_x, W^2)) * 1e-3

Shapes: mu_x/sigma_x [16,128,96,96], W [256,128,5,5], bias [256],
w_sigma [256,1].  Outputs [16,256,92,92] (VALID conv).

Strategy: data-parallel over batch across 8 NeuronCores (2 images/core).

mu conv: direct conv in bf16 (rel err ~1.5e-3, tolerance 2e-2): for each
5-row output block, 25 accumulating matmuls (contraction over C=128 in
partitions) into one PSUM bank; 5 blocks accumulate concurrently in 5
banks so each weight tile serves 5 back-to-back matmuls and the separate
LDWEIGHTS stream hides under the matmuls (fp32r could not do this: its
4-byte self-loading weight path serializes ~107ns into every matmul).

sigma conv (W^2 term): plain fp8 e4m3 matmuls, same 25-tap structure as
the mu conv.  The W^2 term is ~1000x smaller than the softplus box term,
so full fp8 quantization costs only ~2e-6 rel err on sigma_y; fp8 runs
at bf16 rate but quarters the input DMA and gets the fastest weight
loads.  (A DoubleRow-paired variant was tried and wedged the execution
unit nondeterministically -- NRT_EXEC_UNIT_UNRECOVERABLE -- so it was
dropped in favor of this proven instruction mix.)

Box-filter term: channel sums computed directly as two accumulating
matmuls per 5-row chunk: ones(2^-7 fp8) x sigma8 (undoes the 2^7 input
scale exactly) plus ones(fp16) x mu^2(fp16); then a banded matmul for
the vertical 5-box and DVE adds for the horizontal 5-box.  The softplus
term enters each sigma PSUM group as one bf16 rank-1 matmul.  Scales
(2^7 on sigma_x, 2^14 on W^2, 1e-3) are folded into the host-side
weights and the sigma eviction multiply.
"""

import contextlib

import numpy as np
import ml_dtypes

import concourse.bacc as bacc
import concourse.tile as tile
from concourse import mybir
from concourse.ap import AP
from concourse.bass_utils import run_bass_kernel_spmd

F32 = mybir.dt.float32
BF16 = mybir.dt.bfloat16
FP16 = mybir.dt.float16
FP8 = mybir.dt.float8e4

B, C, O, H, W_IN, KK = 16, 128, 256, 96, 96, 5
HO = WO = 92
NCORES = 8
BPC = B // NCORES          # images per core
OCH = O // 128             # output-channel chunks
RB = 5                     # output rows per PSUM group
NFLAT = RB * W_IN          # 480: flat moving free dim for the sigma conv

# 19 output row blocks; the last starts at 87 so it stays full-height
# (rows 87..91), overlapping rows 87..89 of the previous block (benign
# double-write of identical values).
BLOCK_STARTS = [5 * i for i in range(18)] + [HO - RB]
# channel-sum chunks over the 96 input rows: 19 x 5 rows + one final
# 5-row chunk starting at 91 (rows 91..95, overlap rows 91..94).
CS_STARTS = [5 * i for i in range(19)] + [H - RB]
# row-block sets: all blocks in a set accumulate concurrently in distinct
# PSUM banks so one weight load serves the whole set
BLOCK_SETS = [BLOCK_STARTS[i : i + 5] for i in range(0, len(BLOCK_STARTS), 5)]

S_SG = 2.0**7     # host scale on sigma_x before e4m3 quantization
S_W2 = 2.0**14    # host scale on W^2 before e4m3 quantization
ALPHA = 1e-3 / (S_SG * S_W2)   # sigma eviction descale
N_PAIR = 10       # DoubleRow kernel-position pairs (vertical)

_CACHE = {}


def _build(iters=1):
    key = ("nc", iters)
    if key in _CACHE:
        return _CACHE[key]

    nc = bacc.Bacc(None)
    mu_d = nc.dram_tensor("mu", [BPC, C, H, W_IN], BF16, kind="ExternalInput")
    sg_d = nc.dram_tensor("sg", [BPC, C, H, W_IN], FP8, kind="ExternalInput")
    wmu_d = nc.dram_tensor("wmu", [C, OCH, KK * KK, 128], BF16, kind="ExternalInput")
    wsg_d = nc.dram_tensor("wsg", [C, OCH, KK * KK, 128], FP8, kind="ExternalInput")
    bias_d = nc.dram_tensor("bias", [128, OCH], F32, kind="ExternalInput")
    sp_d = nc.dram_tensor("sp", [1, O], BF16, kind="ExternalInput")
    band_d = nc.dram_tensor("band", [H, HO], F32, kind="ExternalInput")
    muy_d = nc.dram_tensor("muy", [BPC, O, HO, WO], F32, kind="ExternalOutput")
    sgy_d = nc.dram_tensor("sgy", [BPC, O, HO, WO], F32, kind="ExternalOutput")

    with tile.TileContext(nc) as tc:
        with (
            tc.tile_pool(name="consts", bufs=1) as consts,
            tc.tile_pool(name="imgs", bufs=1) as imgs,
            tc.tile_pool(name="boxs", bufs=2) as boxs,
            tc.tile_pool(name="ufc", bufs=3) as ufc,
            tc.tile_pool(name="bfc", bufs=4) as bfc,
            tc.tile_pool(name="stag_mu", bufs=3) as stag_mu,
            tc.tile_pool(name="stag_sg", bufs=3) as stag_sg,
            tc.tile_pool(name="ps_conv", bufs=6, space="PSUM") as ps_conv,
            tc.tile_pool(name="ps_u", bufs=1, space="PSUM") as ps_u,
            tc.tile_pool(name="ps_v", bufs=1, space="PSUM") as ps_v,
        ):
            wmu_sb = consts.tile([C, OCH, KK * KK, 128], BF16)
            wsg_sb = consts.tile([C, OCH, KK * KK, 128], FP8)
            bias_sb = consts.tile([128, OCH], F32)
            sp_sb = consts.tile([1, O], BF16)
            band_sb = consts.tile([H, HO], F32)
            ones_sg = consts.tile([C, 1], FP8)
            ones16 = consts.tile([C, 1], FP16)
            nc.sync.dma_start(wmu_sb[:], wmu_d[:])
            nc.sync.dma_start(wsg_sb[:], wsg_d[:])
            nc.sync.dma_start(bias_sb[:], bias_d[:])
            nc.sync.dma_start(sp_sb[:], sp_d[:])
            nc.sync.dma_start(band_sb[:], band_d[:])
            nc.vector.memset(ones_sg[:], 1.0 / S_SG)
            nc.vector.memset(ones16[:], 1.0)

            loop_cm = tc.For_i(0, iters, 1) if iters > 1 else contextlib.nullcontext()
            with loop_cm:
              for img in range(BPC):
                  mu_sb = imgs.tile([C, H, W_IN], BF16, tag="mu")
                  sg_sb = imgs.tile([C, H, W_IN], FP8, tag="sg")
                  nc.sync.dma_start(mu_sb[:], mu_d[img])
                  nc.sync.dma_start(sg_sb[:], sg_d[img])

                  mu2_sb = imgs.tile([C, H, W_IN], FP16, tag="mu2")
                  nc.vector.tensor_mul(mu2_sb[:], mu_sb[:], mu_sb[:])

                  def mu_evict(ps, r0, och):
                      st = stag_mu.tile([128, RB, WO], F32, tag="st")
                      nc.scalar.add(st[:], ps[:], bias_sb[:, och : och + 1])
                      nc.sync.dma_start(
                          muy_d[img, och * 128 : (och + 1) * 128, r0 : r0 + RB, :],
                          st[:],
                      )

                  def sg_evict(ps, r0, och):
                      st = stag_sg.tile([128, RB, WO], F32, tag="st")
                      nc.vector.tensor_scalar_mul(st[:], ps[:], ALPHA)
                      nc.sync.dma_start(
                          sgy_d[img, och * 128 : (och + 1) * 128, r0 : r0 + RB, :],
                          st[:],
                      )

                  def mu_set(och, blocks):
                      pss = [
                          (r0, ps_conv.tile([128, RB, WO], F32, tag="ps", name=f"ps{r0}"))
                          for r0 in blocks
                      ]
                      for ki in range(KK * KK):
                          kh, kw = divmod(ki, KK)
                          for r0, ps in pss:
                              nc.tensor.matmul(
                                  ps[:],
                                  wmu_sb[:, och, ki, :],
                                  mu_sb[:, r0 + kh : r0 + kh + RB, kw : kw + WO],
                                  start=(ki == 0),
                                  stop=(ki == KK * KK - 1),
                              )
                      for r0, ps in pss:
                          mu_evict(ps, r0, och)

                  def sg_set(och, blocks, box2d):
                      bfs = []
                      for r0 in blocks:
                          bf = bfc.tile([1, RB, WO], BF16, tag="bf", name=f"bf{r0}")
                          nc.sync.dma_start(bf[:], box2d[r0 : r0 + RB, :])
                          bfs.append(bf)
                      pss = [
                          (r0, ps_conv.tile([128, RB, WO], F32, tag="ps", name=f"ps{r0}"))
                          for r0 in blocks
                      ]
                      for ki in range(KK * KK):
                          kh, kw = divmod(ki, KK)
                          for r0, ps in pss:
                              nc.tensor.matmul(
                                  ps[:],
                                  wsg_sb[:, och, ki, :],
                                  sg_sb[:, r0 + kh : r0 + kh + RB, kw : kw + WO],
                                  start=(ki == 0),
                                  stop=False,
                              )
                      for (r0, ps), bf in zip(pss, bfs):
                          nc.tensor.matmul(
                              ps[:],
                              sp_sb[0:1, och * 128 : (och + 1) * 128],
                              bf[:],
                              start=False,
                              stop=True,
                          )
                      for r0, ps in pss:
                          sg_evict(ps, r0, och)

                  # ---- mu conv, first output-channel chunk ----
                  for blocks in BLOCK_SETS:
                      mu_set(0, blocks)

                  # ---- channel sums for the box term ----
                  u2d = boxs.tile([H, W_IN], F32, tag="u2d")
                  for cs in CS_STARTS:
                      ups = ps_u.tile([1, NFLAT], F32, tag="ups")
                      nc.tensor.matmul(
                          ups[:], ones_sg[:], sg_sb[:, cs : cs + RB, :],
                          start=True, stop=False,
                      )
                      nc.tensor.matmul(
                          ups[:], ones16[:],
                          mu2_sb[:, cs : cs + RB, :],
                          start=False, stop=True,
                      )
                      uf = ufc.tile([1, NFLAT], F32, tag="uf")
                      nc.scalar.copy(uf[:], ups[:])
                      nc.sync.dma_start(u2d[cs : cs + RB, :], uf[:])

                  # ---- mu conv och1 set 0, then the banded vertical box ----
                  mu_set(1, BLOCK_SETS[0])

                  vb_ps = ps_v.tile([HO, W_IN], F32, tag="vb")
                  nc.tensor.matmul(
                      vb_ps[:], band_sb[:], u2d[:], start=True, stop=True,
                  )
                  vb_sb = boxs.tile([HO, W_IN], F32, tag="vbs")
                  nc.vector.tensor_copy(vb_sb[:], vb_ps[:])
                  # horizontal 5-box accumulated in f32, single round to
                  # bf16; cols 92..95 get finite junk (they only feed
                  # evict-ignored PSUM columns)
                  box32 = boxs.tile([HO, WO], F32, tag="box32")
                  nc.vector.tensor_add(
                      box32[:], vb_sb[:, 0:WO], vb_sb[:, 1 : 1 + WO]
                  )
                  for kw in (2, 3, 4):
                      nc.vector.tensor_add(
                          box32[:], box32[:], vb_sb[:, kw : kw + WO]
                      )
                  box2d = boxs.tile([HO, WO], BF16, tag="box")
                  nc.vector.tensor_copy(box2d[:], box32[:])

                  for blocks in BLOCK_SETS[1:]:
                      mu_set(1, blocks)

                  # ---- sigma conv (both chunks) ----
                  for och in range(OCH):
                      for blocks in BLOCK_SETS:
                          sg_set(och, blocks, box2d)

    nc.compile()
    _CACHE[key] = nc
    return nc


def _host_prep(mu_x, sigma_x, W, bias, w_sigma):
    W = np.asarray(W, dtype=np.float32)
    bias = np.asarray(bias, dtype=np.float32)
    w_sigma = np.asarray(w_sigma, dtype=np.float32)

    mu_bf = np.asarray(mu_x, dtype=np.float32).astype(ml_dtypes.bfloat16)
    sg8 = np.clip(
        np.asarray(sigma_x, dtype=np.float32) * S_SG, -240.0, 240.0
    ).astype(ml_dtypes.float8_e4m3)

    # [o, c, kh, kw] -> [c, och, k, o_in]
    w4 = W.reshape(OCH, 128, C, KK * KK)
    wmu = np.ascontiguousarray(w4.transpose(2, 0, 3, 1)).astype(ml_dtypes.bfloat16)

    w2q = np.clip(
        (W.astype(np.float64) ** 2 * S_W2).astype(np.float32), -240.0, 240.0
    ).astype(ml_dtypes.float8_e4m3)
    wsg8 = np.ascontiguousarray(
        w2q.reshape(OCH, 128, C, KK * KK).transpose(2, 0, 3, 1)
    )

    bias_arr = np.ascontiguousarray(bias.reshape(OCH, 128).T)
    sp = np.log1p(np.exp(np.maximum(w_sigma.astype(np.float64), -88.0)))
    sp_row = np.ascontiguousarray(
        (sp[:, 0] * S_SG * S_W2)[None, :]
    ).astype(ml_dtypes.bfloat16)
    band = np.zeros((H, HO), dtype=np.float32)
    for y2 in range(HO):
        band[y2 : y2 + KK, y2] = 1.0
    return mu_bf, sg8, wmu, wsg8, bias_arr, sp_row, band


def _make_in_maps(mu_x, sigma_x, W, bias, w_sigma):
    mu_bf, sg8, wmu, wsg8, bias_arr, sp_row, band = _host_prep(
        mu_x, sigma_x, W, bias, w_sigma
    )
    in_maps = []
    for c in range(NCORES):
        in_maps.append(
            {
                "mu": mu_bf[c * BPC : (c + 1) * BPC],
                "sg": sg8[c * BPC : (c + 1) * BPC],
                "wmu": wmu,
                "wsg": wsg8,
                "bias": bias_arr,
                "sp": sp_row,
                "band": band,
            }
        )
    return in_maps


def kernel(mu_x, sigma_x, W, bias, w_sigma):
    in_maps = _make_in_maps(mu_x, sigma_x, W, bias, w_sigma)
    nc = _build()
    res = run_bass_kernel_spmd(nc, in_maps, core_ids=list(range(NCORES)))
    mu_y = np.concatenate([res.results[c]["muy"] for c in range(NCORES)], axis=0)
    sigma_y = np.concatenate([res.results[c]["sgy"] for c in range(NCORES)], axis=0)
    return mu_y.astype(np.float32), sigma_y.astype(np.float32)
